# revision 35
# baseline (speedup 1.0000x reference)
"""Multi-head self-attention with RoPE — 8-core SPMD Bass kernel for TRN2 (v2).

Problem: nn_MultiHeadSelfAttention (b=2, s=2048, d=1024, h=16, hd=64),
y = softmax(mask(RoPE(xWq^T) RoPE(xWk^T)^T / 8)) (xWv^T) Wo^T.

Sharding (tensor/head parallel): heads 2i, 2i+1 -> core i. Each core
computes Q/K/V for its 2 heads over the full sequence, applies RoPE,
computes attention, and emits a row-sharded PARTIAL output projection
(Wo columns for its 128 context dims applied to all 1024 output dims):
no collectives — the host sums the 8 partial [1024, 4096] f16 outputs
(collectives measure ~335us/call in this environment vs ~10us tabled,
so the all-reduce is done during host-side unsharding instead).

v2 engineering notes (probe-driven, see session notes):
 - ALL matmuls are K=128 [128,128]x[128,512]. K=64 matmuls sliced at
   partition bases 0/64 (v1's per-head scores) measure 443ns vs 237ns,
   and alternating them with K=128 PV matmuls measures ~1777ns/pair on
   HW (cost model says 426ns). Scores instead use zero-padded per-head
   query operands: qz[head] has the other head's 64 partitions zeroed,
   so the key tile is a full [128,128] stationary operand shared by
   both heads. Probe: uniform-K128 st->exp->PV loop runs 474ns/iter vs
   1119ns for the K=64 version.
 - f16 activations end to end (fp32 PSUM accumulate): halves DMA and
   SBUF, 2x DVE, enables DMA-engine transposes. rel err ~1e-3 << 2e-2.
 - V is transposed to [keys, hd] via DMA-engine xbar transposes (f16),
   not PE transposes: no PSUM bank, no PE time, no DVE copies.
   V tiles are zero-padded per head like qz so head1's attention
   output lands on partitions 64:127 (out-proj rhs needs both heads
   stacked); the softmax denominators ride as a ones column per head
   (row 64 for head0, row 0 for head1).
 - Normalization: DVE reciprocal on the two denominator rows, one K=2
   PE matmul broadcasts them to [128,512], two aligned DVE muls produce
   the normalized f16 out-proj rhs. (v1's gpsimd partition_broadcast
   measured ~45us marginal for 16 calls.)
 - PSUM budget (8 banks): proj 2 + aux(perm/bc/fo shared) 1 + st 3 +
   oc0 1 + oc1 1 = 8. fo matmuls are emitted lagged one query-block so
   the shared aux bank's WAR waits are pre-satisfied.
"""

from contextlib import ExitStack

import numpy as np

import concourse.bacc as bacc_mod
import concourse.tile as tile
from concourse import mybir
from concourse.bass_utils import run_bass_kernel_spmd

F32 = mybir.dt.float32
F32R = mybir.dt.float32r
F16 = mybir.dt.float16
EXP = mybir.ActivationFunctionType.Exp

B = 2
S = 2048
D = 1024
H = 16
HD = 64
NCORES = 8
HPC = H // NCORES          # 2 heads per core
S2 = B * S                 # 4096
NKT = S // 128             # 16 key tiles per batch
NQB = S // 512             # 4 query blocks per batch
DKT = D // 128             # 8 contraction tiles for d=1024
MASK_NEG = -30.0
VW = 258                   # v_all cols: [h0 64 | one | Z64][one | Z63 | h1 64 | pad]


def build_kernel(repeats: int = 1, collectives: bool = True,
                 stages: frozenset = frozenset({"att", "norm", "outproj"})):
    nc = bacc_mod.Bacc("TRN2", target_bir_lowering=False, debug=False,
                       num_devices=NCORES)

    xT = nc.declare_dram_parameter("xT", [D, S2], F16, isOutput=False)
    wq = nc.declare_dram_parameter("wq", [D, 128], F16, isOutput=False)
    wk = nc.declare_dram_parameter("wk", [D, 128], F16, isOutput=False)
    wv = nc.declare_dram_parameter("wv", [D, 128], F16, isOutput=False)
    wo = nc.declare_dram_parameter("wo", [128, D], F16, isOutput=False)
    cosT = nc.declare_dram_parameter("cosT", [128, S2], F16, isOutput=False)
    sinT = nc.declare_dram_parameter("sinT", [128, S2], F16, isOutput=False)
    perm = nc.declare_dram_parameter("perm", [128, 128], F16, isOutput=False)
    sel = nc.declare_dram_parameter("sel", [2, 128], F32R, isOutput=False)
    maskb = nc.declare_dram_parameter("maskb", [128, B * NKT], F32,
                                      isOutput=False)
    onesc = nc.declare_dram_parameter("onesc", [128, 2 * B * NKT], F16,
                                      isOutput=False)
    out = nc.declare_dram_parameter("out", [D, S2], F16, isOutput=True)

    xT_t = xT.rearrange("(kt p) (b c) -> p kt b c", p=128, b=B)
    w_t = {n: w.rearrange("(kt p) m -> p kt m", p=128)
           for n, w in (("wq", wq), ("wk", wk), ("wv", wv))}
    wo_t = wo.rearrange("p (kt m) -> p kt m", m=128)
    cos_t = cosT.rearrange("p (b c) -> p b c", b=B)
    sin_t = sinT.rearrange("p (b c) -> p b c", b=B)
    out_t = out.rearrange("(kt p) c -> p kt c", p=128)

    with tile.TileContext(nc) as tc:
        with nc.allow_low_precision(
                reason="f16 activations; tolerance is 2e-2 rel"):
            for _ in range(repeats):
                _emit_body(nc, tc, xT_t, w_t, wo_t, cos_t, sin_t, perm, sel,
                           maskb, onesc, out_t, stages)
    nc.compile()
    return nc


def _emit_body(nc, tc, xT_t, w_t, wo_t, cos_t, sin_t, perm, sel, maskb, onesc,
               out_t, stages):
    with ExitStack() as body:
        consts = body.enter_context(tc.tile_pool(name="consts", bufs=1))
        w_sb = {}
        for n in ("wq", "wk", "wv"):
            w_sb[n] = consts.tile([128, DKT, 128], F16, name=f"{n}_sb")
            nc.sync.dma_start(out=w_sb[n], in_=w_t[n])
        w_sb["wo"] = consts.tile([128, DKT, 128], F16, name="wo_sb")
        nc.sync.dma_start(out=w_sb["wo"], in_=wo_t)
        perm_sb = consts.tile([128, 128], F16)
        nc.sync.dma_start(out=perm_sb, in_=perm[:, :])
        sel_sb = consts.tile([1, 2, 128], F32R)
        nc.sync.dma_start(out=sel_sb,
                          in_=sel.rearrange("(o t) m -> o t m", o=1))
        maskb_sb = consts.tile([128, B * NKT], F32)
        nc.sync.dma_start(out=maskb_sb, in_=maskb[:, :])

        # persistent activations
        acts = body.enter_context(tc.tile_pool(name="acts", bufs=1))
        krot = acts.tile([128, B, S], F16)
        v_all = acts.tile([128, B, NKT, VW], F16)
        # ones columns (denominator riders) + zero pad inside head1 tiles
        ones_r = onesc.rearrange("p (t b k) -> p t b k", t=2, b=B)
        nc.sync.dma_start(out=v_all[:, :, :, 64], in_=ones_r[:, 0])
        nc.sync.dma_start(out=v_all[:, :, :, 129], in_=ones_r[:, 1])
        nc.vector.memset(v_all[:, :, :, 130:193], 0.0)

        # PSUM: pr 2 + aux 1 + st 3 + oc0 1 + oc1 1 = 8 banks
        pr_ps = body.enter_context(
            tc.tile_pool(name="pr_ps", bufs=2, space="PSUM"))
        aux_ps = body.enter_context(
            tc.tile_pool(name="aux_ps", bufs=1, space="PSUM"))
        st_ps = body.enter_context(
            tc.tile_pool(name="st_ps", bufs=3, space="PSUM"))
        oc_ps = body.enter_context(
            tc.tile_pool(name="oc_ps", bufs=1, space="PSUM"))

        xpool = body.enter_context(tc.tile_pool(name="xpool", bufs=2))
        cpool = body.enter_context(tc.tile_pool(name="cpool", bufs=2))
        tmp = body.enter_context(tc.tile_pool(name="tmp", bufs=3))
        vtp = body.enter_context(tc.tile_pool(name="vtp", bufs=2))
        qzp = body.enter_context(tc.tile_pool(name="qzp", bufs=2))
        ppool = body.enter_context(tc.tile_pool(name="ppool", bufs=8))
        npool = body.enter_context(tc.tile_pool(name="npool", bufs=2))
        upool = body.enter_context(tc.tile_pool(name="upool", bufs=2))
        opool = body.enter_context(tc.tile_pool(name="opool", bufs=2))

        xsb = {}
        cos_sb = {}
        sin_sb = {}
        qz = {}
        oc = {}
        u_sb = {}

        def emit_xload(b):
            xsb[b] = xpool.tile([128, DKT, S], F16, tag="xsb",
                                name=f"xsb{b}")
            for half in range(2):
                nc.sync.dma_start(
                    out=xsb[b][:, 4 * half:4 * half + 4, :],
                    in_=xT_t[:, 4 * half:4 * half + 4, b, :])
            cos_sb[b] = cpool.tile([128, S], F16, tag="cos", name=f"cos{b}")
            sin_sb[b] = cpool.tile([128, S], F16, tag="sin", name=f"sin{b}")
            nc.sync.dma_start(out=cos_sb[b], in_=cos_t[:, b, :])
            nc.sync.dma_start(out=sin_sb[b], in_=sin_t[:, b, :])

        def emit_proj(b, c, name):
            """Project block c of batch b through w[name] -> f16 raw tile."""
            pr = pr_ps.tile([128, 512], F32, tag="pr", name="pr")
            for kt in range(DKT):
                nc.tensor.matmul(pr, w_sb[name][:, kt, :],
                                 xsb[b][:, kt, c * 512:(c + 1) * 512],
                                 start=(kt == 0), stop=(kt == DKT - 1))
            raw = tmp.tile([128, 512], F16, tag="raw", name="raw")
            nc.vector.tensor_copy(raw, pr)
            return raw

        def emit_rope(b, c, raw):
            """-> (tcos, tsin) f16 [128,512] to be added per-destination."""
            pp = aux_ps.tile([128, 512], F32, tag="aux", name="pp")
            nc.tensor.matmul(pp, perm_sb, raw, start=True, stop=True)
            tcos = tmp.tile([128, 512], F16, tag="tcos", name="tcos")
            nc.vector.tensor_mul(tcos, raw,
                                 cos_sb[b][:, c * 512:(c + 1) * 512])
            tsin = tmp.tile([128, 512], F16, tag="tsin", name="tsin")
            nc.vector.tensor_mul(tsin, pp,
                                 sin_sb[b][:, c * 512:(c + 1) * 512])
            return tcos, tsin

        def emit_K(b, c):
            raw = emit_proj(b, c, "wk")
            tcos, tsin = emit_rope(b, c, raw)
            nc.vector.tensor_add(krot[:, b, c * 512:(c + 1) * 512],
                                 tcos, tsin)

        def emit_V(b, c):
            rawv = emit_proj(b, c, "wv")
            # one tiled DMA xbar transpose (key j -> tile j//128, partition
            # j%128), then aligned free-dim copies into the padded v layout
            vstage = vtp.tile([128, 4, 128], F16, tag="vstage",
                              name="vstage")
            nc.sync.dma_start(out=vstage, in_=rawv, transpose=True)
            for i in range(4):
                kt = 4 * c + i
                nc.vector.tensor_copy(v_all[:, b, kt, 0:64],
                                      vstage[:, i, 0:64])
                nc.vector.tensor_copy(v_all[:, b, kt, 193:257],
                                      vstage[:, i, 64:128])

        def emit_Q(b, qb):
            raw = emit_proj(b, qb, "wq")
            tcos, tsin = emit_rope(b, qb, raw)
            for ln in range(HPC):
                qzt = qzp.tile([128, 512], F16, tag=f"qz{ln}",
                               name=f"qz{ln}")
                qz[(b, qb, ln)] = qzt
                nc.vector.memset(qzt[(1 - ln) * 64:(2 - ln) * 64, :], 0.0)
                nc.vector.tensor_add(qzt[ln * 64:(ln + 1) * 64, :],
                                     tcos[ln * 64:(ln + 1) * 64, :],
                                     tsin[ln * 64:(ln + 1) * 64, :])

        def emit_st(b, qb, kt):
            """Score+exp for both heads of key tile kt -> (p0, p1)."""
            mb = maskb_sb[:, (b * NKT + kt):(b * NKT + kt) + 1]
            ps = []
            for ln in range(HPC):
                st = st_ps.tile([128, 512], F32, tag="st", name="st")
                nc.tensor.matmul(st, krot[:, b, kt * 128:(kt + 1) * 128],
                                 qz[(b, qb, ln)], start=True, stop=True)
                p = ppool.tile([128, 512], F16, tag="p", name="p")
                nc.scalar.activation(p, st, EXP, bias=mb, scale=1.0)
                ps.append(p)
            return ps

        def emit_oc(b, qb, kt, ps):
            for ln in range(HPC):
                if kt == 0:
                    shape = [65, 512] if ln == 0 else [128, 512]
                    oc[(b, qb, ln)] = oc_ps.tile(shape, F32, tag=f"oc{ln}",
                                                 name=f"oc{ln}")
                vsl = (v_all[:, b, kt, 0:65] if ln == 0
                       else v_all[:, b, kt, 129:257])
                nc.tensor.matmul(oc[(b, qb, ln)], vsl, ps[ln],
                                 start=(kt == 0), stop=(kt == NKT - 1))

        def emit_norm(b, qb):
            oc0 = oc[(b, qb, 0)]
            oc1 = oc[(b, qb, 1)]
            rec0 = npool.tile([1, 512], F32R, tag="rec0", name="rec0")
            nc.vector.reciprocal(rec0, oc0[64:65, :])
            rec1 = npool.tile([1, 512], F32R, tag="rec1", name="rec1")
            nc.vector.reciprocal(rec1, oc1[0:1, :])
            bc = aux_ps.tile([128, 512], F32, tag="aux", name="bc")
            nc.tensor.matmul(bc, sel_sb[:, 0, :], rec0, start=True,
                             stop=False)
            nc.tensor.matmul(bc, sel_sb[:, 1, :], rec1, start=False,
                             stop=True)
            bcs = npool.tile([128, 512], F32, tag="bcs", name="bcs")
            nc.vector.tensor_copy(bcs, bc)
            u = upool.tile([128, 512], F16, tag="u", name="u")
            u_sb[(b, qb)] = u
            import os
            if os.environ.get("SKIP_NORM"):
                nc.vector.tensor_copy(u[0:64, :], oc0[0:64, :])
                nc.vector.tensor_copy(u[64:128, :], oc1[64:128, :])
            else:
                nc.vector.tensor_mul(u[0:64, :], oc0[0:64, :], bcs[0:64, :])
                nc.vector.tensor_mul(u[64:128, :], oc1[64:128, :],
                                     bcs[64:128, :])

        osb8 = {}

        def emit_fo_one(b, qb, kt):
            if kt == 0:
                osb8[(b, qb)] = opool.tile([128, DKT, 512], F16, tag="osb",
                                           name="osb")
            fo = aux_ps.tile([128, 512], F32, tag="aux", name="fo")
            nc.tensor.matmul(fo, w_sb["wo"][:, kt, :], u_sb[(b, qb)],
                             start=True, stop=True)
            nc.vector.tensor_copy(osb8[(b, qb)][:, kt, :], fo)
            if kt == DKT - 1:
                nc.sync.dma_start(
                    out=out_t[:, :, b * S + qb * 512:b * S + (qb + 1) * 512],
                    in_=osb8[(b, qb)])

        def emit_att(b, qb, interleave):
            """16 kt chunks, software-pipelined (oc lags st/exp by one key
            tile so PV waits are pre-satisfied), with interleaved work items
            from `interleave`: a dict {kt: [callable, ...]}."""
            prev = None
            for kt in range(NKT):
                ps = emit_st(b, qb, kt)
                if prev is not None:
                    emit_oc(b, qb, kt - 1, prev)
                prev = ps
                for fn in interleave.get(kt, ()):
                    fn()
            emit_oc(b, qb, NKT - 1, prev)
            emit_norm(b, qb)

        def stub_out():
            osb0 = opool.tile([128, 512], F16, tag="stub", name="stub")
            src = u_sb[(0, 0)] if (0, 0) in u_sb else krot[:, 0, 0:512]
            nc.vector.tensor_copy(osb0, src)
            nc.sync.dma_start(out=out_t[:, 0, 0:512], in_=osb0)

        # ---------------- schedule ----------------
        emit_xload(0)
        for c in range(NQB):
            emit_K(0, c)
            emit_V(0, c)
        if "att" not in stages:
            emit_xload(1)
            for c in range(NQB):
                emit_K(1, c)
                emit_V(1, c)
            emit_Q(0, 0)
            emit_Q(1, 0)
            stub_out()
            return

        emit_Q(0, 0)
        pend = []           # deferred fo emissions (one per (b, qb))

        def make_fo_items(b, qb):
            return [lambda kt=kt: emit_fo_one(b, qb, kt)
                    for kt in range(DKT)]

        for b in range(B):
            for qb in range(NQB):
                inter = {}
                nxt = (b, qb + 1) if qb + 1 < NQB else (
                    (b + 1, 0) if b + 1 < B else None)
                if qb == NQB - 2 and b + 1 < B:
                    # start next batch's x/cos/sin DMAs one block early
                    inter[8] = [lambda bb=b + 1: emit_xload(bb)]
                if nxt is not None:
                    if nxt[1] == 0:
                        for cc in range(4):
                            inter.setdefault(2 + 3 * cc, []).append(
                                lambda bb=nxt[0], c2=cc: emit_K(bb, c2))
                            inter.setdefault(3 + 3 * cc, []).append(
                                lambda bb=nxt[0], c2=cc: emit_V(bb, c2))
                        inter.setdefault(13, []).append(
                            lambda bb=nxt[0]: emit_Q(bb, 0))
                    else:
                        inter[2] = [lambda bb=b, qq=qb + 1: emit_Q(bb, qq)]
                # spread pending fo matmuls through chunks 4..11
                if pend:
                    items = pend.pop(0)
                    for i, fn in enumerate(items):
                        inter.setdefault(4 + i, []).append(fn)
                emit_att(b, qb, inter)
                if "outproj" in stages:
                    pend.append(make_fo_items(b, qb))
        # drain remaining fo work (last query block)
        for items in pend:
            for fn in items:
                fn()
        if "outproj" not in stages:
            stub_out()


# ---------------- host-side shard prep / unshard ----------------

def prep_inputs(x, attn_mask, Wq, Wk, Wv, Wo):
    """Full inputs -> list of 8 per-core input dicts."""
    x = np.asarray(x, dtype=np.float32)
    Wq = np.asarray(Wq, dtype=np.float32)
    Wk = np.asarray(Wk, dtype=np.float32)
    Wv = np.asarray(Wv, dtype=np.float32)
    Wo = np.asarray(Wo, dtype=np.float32)
    attn_mask = np.asarray(attn_mask)

    xT = np.ascontiguousarray(x.reshape(S2, D).T.astype(np.float16))

    # deinterleave: even hd components then odd, within each head
    comp = np.concatenate([np.arange(0, HD, 2), np.arange(1, HD, 2)])  # [64]
    half = HD // 2
    pi = np.concatenate([np.arange(half), np.arange(half)])            # [64]
    freq = np.float32(10000.0) ** (-2.0 * pi.astype(np.float32) / HD)
    pos = np.arange(S, dtype=np.float32)
    ang = pos[None, :] * freq[:, None]                     # [64, 2048]
    cos1 = np.cos(ang).astype(np.float16)
    sin1 = np.sin(ang).astype(np.float16)
    cosT = np.ascontiguousarray(
        np.tile(np.concatenate([cos1, cos1], axis=0), (1, B)))  # [128, 4096]
    sinT = np.ascontiguousarray(
        np.tile(np.concatenate([sin1, sin1], axis=0), (1, B)))

    permM = np.zeros((128, 128), dtype=np.float16)   # perm[p_in, p_out]
    for ln in range(HPC):
        base = ln * 64
        for j in range(half):
            permM[base + half + j, base + j] = -1.0
            permM[base + j, base + half + j] = 1.0

    selM = np.zeros((2, 128), dtype=np.float32)
    selM[0, 0:64] = 1.0
    selM[1, 64:128] = 1.0

    maskbM = np.zeros((128, B * NKT), dtype=np.float32)
    for b in range(B):
        for kt in range(NKT):
            mslice = attn_mask[b, kt * 128:(kt + 1) * 128]
            maskbM[:, b * NKT + kt] = np.where(
                mslice, np.float32(MASK_NEG), 0.0)

    onescM = np.ones((128, 2 * B * NKT), dtype=np.float16)

    in_maps = []
    for i in range(NCORES):
        heads = [HPC * i + ln for ln in range(HPC)]
        rows_qk = np.concatenate([h * HD + comp for h in heads])      # [128]
        rows_v = np.concatenate(
            [np.arange(h * HD, (h + 1) * HD) for h in heads])
        wq_i = np.ascontiguousarray(
            (Wq[rows_qk, :] / 8.0).T.astype(np.float16))    # [1024, 128]
        wk_i = np.ascontiguousarray(Wk[rows_qk, :].T.astype(np.float16))
        wv_i = np.ascontiguousarray(Wv[rows_v, :].T.astype(np.float16))
        wo_i = np.ascontiguousarray(
            Wo[:, rows_v].T.astype(np.float16))             # [128, 1024]
        in_maps.append({
            "xT": xT, "wq": wq_i, "wk": wk_i, "wv": wv_i, "wo": wo_i,
            "cosT": cosT, "sinT": sinT, "perm": permM, "sel": selM,
            "maskb": maskbM, "onesc": onescM,
        })
    return in_maps


def assemble_output(results):
    """list of per-core result dicts -> full [B, S, D] output (sum of
    row-sharded partial projections)."""
    acc = np.zeros((D, S2), dtype=np.float32)
    for i in range(NCORES):
        acc += results[i]["out"].astype(np.float32)
    # acc[o, b*S+s] -> out[b, s, o]
    return np.ascontiguousarray(acc.reshape(D, B, S).transpose(1, 2, 0))


_NC_CACHE = {}


def kernel(x, attn_mask, Wq, Wk, Wv, Wo):
    """Full-input, full-output entry point (shards across 8 NeuronCores)."""
    if "nc" not in _NC_CACHE:
        _NC_CACHE["nc"] = build_kernel()
    nc = _NC_CACHE["nc"]
    in_maps = prep_inputs(x, attn_mask, Wq, Wk, Wv, Wo)
    res = run_bass_kernel_spmd(nc, in_maps, core_ids=list(range(NCORES)))
    return assemble_output(res.results)
